# revision 1
# baseline (speedup 1.0000x reference)
"""Coordinate-descent (alternating Gauss-Seidel) kernel for Trainium2.

Problem: B=4 independent factorizations x ~ u @ v^T with M=N=4096, R=32.
  u_new = GS-sweep(a1 = x@v,   b1 = v^T v, u)
  v_new = GS-sweep(a2 = x^T@u_new, b2 = u_new^T u_new, v)

Sharding: 8 cores, each owns rows [c*512,(c+1)*512) of ALL 4 batches for u,
and the same index range of n-rows for v (delivered by a per-batch
ReduceScatter of the partial a2/b2 products).

Core pipeline (per batch):
  phase1: stream x fp32 tiles -> cast bf16 (kept in SBUF for phase 2),
          PE-transpose -> x^T tiles, a1T = sum_c v_c^T @ xT_c  (bf16 MMs)
  u-GS:   s-incremental Gauss-Seidel sweep on DVE (fp32)
  phase2: a2_partial natural = sum_i x_nat_i^T @ u_new_i (bf16 MMs,
          x natural as stationary), b2_partial; ReduceScatter over 8 cores
  v-GS:   same sweep on the scattered slice
"""

import os
from contextlib import ExitStack

import numpy as np

import concourse.bass as bass
import concourse.tile as tile
from concourse import bacc, mybir
from concourse.bass import ds
from concourse.bass_utils import run_bass_kernel_spmd
from concourse.masks import make_identity

B, M, N, R = 4, 4096, 4096, 32
NCORES = 8
MS = M // NCORES          # 512 rows per core per batch
MC = MS // 128            # 4 m-chunks of 128
NG = N // 512             # 8 n-groups of 512
NCH = N // 128            # 32 n-chunks of 128
EPS = 1e-8
FP32 = mybir.dt.float32
BF16 = mybir.dt.bfloat16
ALU = mybir.AluOpType

_CACHE = {}
LAST_RESULT = None


def _bcast(ap1, ap2):
    return bass.broadcast_tensor_aps(ap1, ap2)


def _gs_prep_and_sweep(nc, gsp, pmisc, ident_f, w32, a_nat, b_sb, brep, wnew):
    """Gauss-Seidel sweep: wnew = sweep(a_nat, b, w32).

    w32:  [128, MC, R] fp32 current factor rows (natural)
    a_nat:[128, MC, R] fp32 (x@other) rows
    b_sb: [R, R] fp32 gram matrix (natural, SBUF)
    brep: [128, R, R] fp32 gram replicated on every partition
    wnew: [128, MC, R] fp32 output AP
    """
    # w^T via PE transposes
    puT = pmisc.tile([R, MC, 128], FP32, tag="pm")
    for i in range(MC):
        nc.tensor.transpose(puT[:, i], w32[:, i], ident_f)
    uT = gsp.tile([R, MC, 128], FP32, tag="uT")
    nc.vector.tensor_copy(uT[:], puT[:])

    # s = w @ b  (contraction over r via K=32 matmuls)
    ps = pmisc.tile([128, MC, R], FP32, tag="pm")
    for i in range(MC):
        nc.tensor.matmul(ps[:, i], lhsT=uT[:, i], rhs=b_sb[:], start=True,
                         stop=True)
    s = gsp.tile([128, MC, R], FP32, tag="s")
    nc.vector.tensor_copy(s[:], ps[:])

    # brr = diag(b) per partition; inv = 1/(brr+eps); app = (a+eps)*inv
    brr = gsp.tile([128, 1, R], FP32, tag="brr")
    diag_ap = bass.AP(brep.tensor, brep[:].offset,
                      [brep[:].ap[0], [R + 1, R]])
    nc.vector.tensor_copy(brr[:, 0], diag_ap)
    inv = gsp.tile([128, 1, R], FP32, tag="inv")
    nc.vector.tensor_scalar_add(inv[:], brr[:], EPS)
    nc.vector.reciprocal(inv[:], inv[:])

    app = gsp.tile([128, MC, R], FP32, tag="app")
    a_bc, inv_bc = _bcast(a_nat[:], inv[:])
    nc.vector.scalar_tensor_tensor(out=app[:], in0=a_bc, scalar=EPS,
                                   in1=inv_bc, op0=ALU.add, op1=ALU.mult)

    t2 = gsp.tile([128, MC], FP32, tag="t2")
    delta = gsp.tile([128, MC, 1], FP32, tag="delta")
    tmp = gsp.tile([128, MC, R - 1], FP32, tag="tmp")
    for r in range(R):
        # t2 = w_r * b_rr - s_r
        nc.vector.scalar_tensor_tensor(
            out=t2[:], in0=w32[:, :, r], scalar=brr[:, 0, r:r + 1],
            in1=s[:, :, r], op0=ALU.mult, op1=ALU.subtract)
        # wnew_r = t2 * inv_r + app_r
        nc.vector.scalar_tensor_tensor(
            out=wnew[:, :, r], in0=t2[:], scalar=inv[:, 0, r:r + 1],
            in1=app[:, :, r], op0=ALU.mult, op1=ALU.add)
        if r < R - 1:
            tail = R - 1 - r
            nc.vector.tensor_tensor(out=delta[:, :, 0], in0=wnew[:, :, r],
                                    in1=w32[:, :, r], op=ALU.subtract)
            d_bc, brow_bc = _bcast(delta[:], brep[:, r:r + 1, r + 1:])
            nc.vector.tensor_tensor(out=tmp[:, :, :tail], in0=d_bc,
                                    in1=brow_bc, op=ALU.mult)
            nc.vector.tensor_tensor(out=s[:, :, r + 1:], in0=s[:, :, r + 1:],
                                    in1=tmp[:, :, :tail], op=ALU.add)


def _build():
    nc = bacc.Bacc("TRN2", target_bir_lowering=False, debug=False,
                   num_devices=NCORES)

    x_my = nc.dram_tensor("x_my", [B, MS, N], FP32, kind="ExternalInput").ap()
    u_my = nc.dram_tensor("u_my", [B, MS, R], FP32, kind="ExternalInput").ap()
    v_full = nc.dram_tensor("v_full", [B, N, R], FP32,
                            kind="ExternalInput").ap()
    v_my = nc.dram_tensor("v_my", [B, MS, R], FP32, kind="ExternalInput").ap()
    u_out = nc.dram_tensor("u_out", [B, MS, R], FP32,
                           kind="ExternalOutput").ap()
    v_out = nc.dram_tensor("v_out", [B, MS, R], FP32,
                           kind="ExternalOutput").ap()

    rs_in = [nc.dram_tensor(f"rs_in{b}", [NCORES * 544, R], FP32)
             for b in range(B)]
    rs_out = [nc.dram_tensor(f"rs_out{b}", [544, R], FP32) for b in range(B)]
    b1_scr = nc.dram_tensor("b1_scr", [B, R, R], FP32)

    with tile.TileContext(nc) as tc, ExitStack() as ctx:
        const = ctx.enter_context(tc.tile_pool(name="const", bufs=1))
        big = ctx.enter_context(tc.tile_pool(name="big", bufs=1))
        xl = ctx.enter_context(tc.tile_pool(name="xl", bufs=4))
        xt = ctx.enter_context(tc.tile_pool(name="xt", bufs=2))
        vpool = ctx.enter_context(tc.tile_pool(name="vp", bufs=2))
        brp = ctx.enter_context(tc.tile_pool(name="brp", bufs=2))
        gsp = ctx.enter_context(tc.tile_pool(name="gsp", bufs=2))
        sm = ctx.enter_context(tc.tile_pool(name="sm", bufs=2))
        ppt = ctx.enter_context(tc.tile_pool(name="ppt", bufs=2, space="PSUM"))
        pa1p = ctx.enter_context(tc.tile_pool(name="pa1", bufs=2,
                                              space="PSUM"))
        pa2p = ctx.enter_context(tc.tile_pool(name="pa2", bufs=2,
                                              space="PSUM"))
        pmisc = ctx.enter_context(tc.tile_pool(name="pmisc", bufs=2,
                                               space="PSUM"))

        ident_b = const.tile([128, 128], BF16)
        make_identity(nc, ident_b)
        ident_f = const.tile([128, 128], FP32)
        make_identity(nc, ident_f)

        x_nat = big.tile([128, B * MC, N], BF16)      # 16.8 MB persistent
        unew_all = big.tile([128, B * MC, R], FP32)

        for b in range(B):
            # ---------- v load + b1 = v^T v ----------
            v32 = vpool.tile([128, NCH, R], FP32, tag="v32")
            nc.sync.dma_start(v32[:],
                              v_full[b].rearrange("(c p) r -> p c r", p=128))
            vb = vpool.tile([128, NCH, R], BF16, tag="vb")
            nc.vector.tensor_copy(vb[:], v32[:])

            pb1 = pmisc.tile([R, R], FP32, tag="pm")
            for c in range(NCH):
                nc.tensor.matmul(pb1[:], lhsT=vb[:, c], rhs=vb[:, c],
                                 start=(c == 0), stop=(c == NCH - 1))
            b1_sb = sm.tile([R, R], FP32, tag="b1")
            nc.vector.tensor_copy(b1_sb[:], pb1[:])
            nc.sync.dma_start(b1_scr.ap()[b], b1_sb[:])
            brep1 = brp.tile([128, R, R], FP32, tag="brep")
            src = b1_scr.ap()[b]
            nc.sync.dma_start(
                brep1[:], bass.AP(src.tensor, src.offset,
                                  [[0, 128], [R, R], [1, R]]))

            # ---------- phase 1: stream x, transpose, a1T ----------
            pa1 = pa1p.tile([R, MS], FP32, tag="pa1")
            for j in range(NG):
                xT = xt.tile([128, 4, MS], BF16, tag="xT")
                for i in range(MC):
                    xload = xl.tile([128, 512], FP32, tag="xload")
                    nc.sync.dma_start(
                        xload[:],
                        x_my[b, i * 128:(i + 1) * 128, j * 512:(j + 1) * 512])
                    nc.scalar.copy(
                        x_nat[:, b * MC + i, j * 512:(j + 1) * 512], xload[:])
                    pt = ppt.tile([128, 4, 128], BF16, tag="pt")
                    for k in range(4):
                        nc.tensor.transpose(
                            pt[:, k],
                            x_nat[:, b * MC + i,
                                  (j * 4 + k) * 128:(j * 4 + k + 1) * 128],
                            ident_b)
                    nc.scalar.copy(xT[:, :, i * 128:(i + 1) * 128], pt[:])
                for k in range(4):
                    c = j * 4 + k
                    nc.tensor.matmul(pa1[:], lhsT=vb[:, c], rhs=xT[:, k],
                                     start=(c == 0), stop=(c == NCH - 1))
            a1T_sb = sm.tile([R, MS], FP32, tag="a1T")
            nc.vector.tensor_copy(a1T_sb[:], pa1[:])

            # ---------- u GS ----------
            u32 = gsp.tile([128, MC, R], FP32, tag="u32")
            nc.sync.dma_start(u32[:],
                              u_my[b].rearrange("(i p) r -> p i r", p=128))
            pA = pmisc.tile([128, MC, R], FP32, tag="pm")
            for i in range(MC):
                nc.tensor.transpose(pA[:, i],
                                    a1T_sb[:, i * 128:(i + 1) * 128],
                                    ident_f[:R, :R])
            a_nat = gsp.tile([128, MC, R], FP32, tag="a_nat")
            nc.vector.tensor_copy(a_nat[:], pA[:])

            unew = unew_all[:, b * MC:(b + 1) * MC, :]
            _gs_prep_and_sweep(nc, gsp, pmisc, ident_f, u32, a_nat, b1_sb,
                               brep1, unew)
            nc.sync.dma_start(u_out[b].rearrange("(i p) r -> p i r", p=128),
                              unew)

            # ---------- phase 2: a2 partial (natural), b2 partial ----------
            un_b = sm.tile([128, MC, R], BF16, tag="unb")
            nc.vector.tensor_copy(un_b[:], unew)

            for g in range(NG):
                pa2 = pa2p.tile([128, 4, R], FP32, tag="pa2")
                for k in range(4):
                    nblk = g * 4 + k
                    for i in range(MC):
                        nc.tensor.matmul(
                            pa2[:, k],
                            lhsT=x_nat[:, b * MC + i,
                                       nblk * 128:(nblk + 1) * 128],
                            rhs=un_b[:, i], start=(i == 0), stop=(i == MC - 1))
                a2st = sm.tile([128, 4, R], FP32, tag="a2st")
                nc.vector.tensor_copy(a2st[:], pa2[:])
                dst = rs_in[b].ap()
                nc.sync.dma_start(
                    bass.AP(dst.tensor, dst.offset + g * 544 * R,
                            [[R, 128], [128 * R, 4], [1, R]]),
                    a2st[:])

            pb2 = pmisc.tile([R, R], FP32, tag="pm")
            for i in range(MC):
                nc.tensor.matmul(pb2[:], lhsT=un_b[:, i], rhs=un_b[:, i],
                                 start=(i == 0), stop=(i == MC - 1))
            b2st = sm.tile([R, R], FP32, tag="b2st")
            nc.vector.tensor_copy(b2st[:], pb2[:])
            for c in range(NCORES):
                nc.sync.dma_start(rs_in[b].ap()[ds(c * 544 + 512, R), :],
                                  b2st[:])

            nc.gpsimd.collective_compute(
                "ReduceScatter", ALU.add,
                replica_groups=[list(range(NCORES))],
                ins=[rs_in[b].ap()], outs=[rs_out[b].ap()])

        # ---------- v GS per batch ----------
        for b in range(B):
            a2my = gsp.tile([128, MC, R], FP32, tag="a_nat")
            nc.sync.dma_start(
                a2my[:],
                rs_out[b].ap()[ds(0, MS), :].rearrange("(i p) r -> p i r",
                                                       p=128))
            b2_sb = sm.tile([R, R], FP32, tag="b1")
            nc.sync.dma_start(b2_sb[:], rs_out[b].ap()[ds(512, R), :])
            brep2 = brp.tile([128, R, R], FP32, tag="brep")
            src = rs_out[b].ap()
            nc.sync.dma_start(
                brep2[:], bass.AP(src.tensor, src.offset + 512 * R,
                                  [[0, 128], [R, R], [1, R]]))

            v32my = gsp.tile([128, MC, R], FP32, tag="u32")
            nc.sync.dma_start(v32my[:],
                              v_my[b].rearrange("(i p) r -> p i r", p=128))
            vnew = gsp.tile([128, MC, R], FP32, tag="vnew")
            _gs_prep_and_sweep(nc, gsp, pmisc, ident_f, v32my, a2my, b2_sb,
                               brep2, vnew[:])
            nc.sync.dma_start(v_out[b].rearrange("(i p) r -> p i r", p=128),
                              vnew[:])

    nc.compile()
    return nc


def kernel(x, u, v):
    global LAST_RESULT
    if "nc" not in _CACHE:
        _CACHE["nc"] = _build()
    nc = _CACHE["nc"]

    x = np.ascontiguousarray(x, dtype=np.float32)
    u = np.ascontiguousarray(u, dtype=np.float32)
    v = np.ascontiguousarray(v, dtype=np.float32)

    in_maps = []
    for c in range(NCORES):
        sl = slice(c * MS, (c + 1) * MS)
        in_maps.append({
            "x_my": np.ascontiguousarray(x[:, sl, :]),
            "u_my": np.ascontiguousarray(u[:, sl, :]),
            "v_full": v,
            "v_my": np.ascontiguousarray(v[:, sl, :]),
        })

    res = run_bass_kernel_spmd(nc, in_maps, list(range(NCORES)),
                               trace=os.environ.get("KBENCH_TRACE") == "1")
    LAST_RESULT = res
    u_new = np.concatenate([res.results[c]["u_out"] for c in range(NCORES)],
                           axis=1)
    v_new = np.concatenate([res.results[c]["v_out"] for c in range(NCORES)],
                           axis=1)
    return (u_new, v_new)
